# revision 1
# baseline (speedup 1.0000x reference)
"""Causal self-attention (dense transformer) on 8 trn2 NeuronCores — v3.

Same core sharding as v1 (batch x row-quarter/head-group per core, host-side
row-parallel reduction of the output projection), with:
- bf16 matmul operands on every DMA-touched tensor (x, weights, V bounce,
  y, output partials); SBUF-resident Q^T/K^T stay f32r, which costs the
  same on the PE (1 cyc/row at >=256 free) but avoids the device's
  truncating fp32->bf16 conversions.  End-to-end error ~3.4e-3.
- QK bias folded into the PSUM eviction on ACT (Identity+bias, per-partition
  bias AP) instead of PE bias matmuls
- V bias added during the V eviction (tensor_tensor add against a
  partition-broadcast bias tile).  NOTE: it cannot be folded into bproj --
  under the headless reshape the v-bias seen by (t, d) is
  bqkv[2048 + 64*(t%16) + d], which varies across the keys averaged by the
  softmax.
- projection bias added on the host (the device emits partial sums)
- softmax denominator reciprocal read straight out of PSUM on DVE; the
  [1,512] -> [64,512] broadcast on Pool (partition_broadcast) instead of a
  PE outer product
- causal masks as 4 precomputed bf16 0/1 tiles (one per diagonal offset),
  applied on DVE (tensor_tensor mult, all-SBUF 2x mode) instead of
  gpsimd affine_select per tile
- V re-partition bounce through DRAM in bf16 with per-head gathers issued
  as soon as each head's V columns are evicted; ones column via memset
- phase 2 runs j blocks in order [3, 2, 0, 1]: the largest block last gives
  the longest S/PV stream to hide the drained closure backlog, its head
  pairs are interleaved so the two trailing norm chains overlap, and the
  final projections split their yt[0] halves into the norm-latency window
- PE warm-up dummy matmuls burn the p-state ramp during the first loads

Shapes (hardcoded): B=2, T=2048, C=1024, n_head=16, hd=64, 8 cores.
Core c: batch b=c//4, quarter q=c%4 -> x rows [512q, 512q+512), heads 4q..4q+3.
"""

import os

import numpy as np
import ml_dtypes

os.environ.setdefault("NEURON_RT_RESET_CORES", "1")

import concourse.bacc as bacc
import concourse.mybir as mybir
import concourse.tile as tile
from concourse.bass_utils import run_bass_kernel_spmd

dt = mybir.dt
AF = mybir.ActivationFunctionType
OP = mybir.AluOpType

B, T, C = 2, 2048, 1024
NH, HD = 16, 64
N_CORES = 8
HPC = 4          # heads per core
RPC = 512        # x rows per core
SCALE = 1.0 / 8.0   # 1/sqrt(hd), folded into the exp activation
BF = dt.bfloat16


def build_program():
    nc = bacc.Bacc("TRN2", target_bir_lowering=False, debug=False,
                   num_devices=N_CORES)

    # ---- DRAM I/O (per core) ----
    xT = nc.dram_tensor("xT", [128, 8, RPC], BF, kind="ExternalInput")
    wq = nc.dram_tensor("wq", [16, 128, 8 * 128], BF, kind="ExternalInput")
    wv = nc.dram_tensor("wv", [2, 128, 8 * 512], BF, kind="ExternalInput")
    bqkT = nc.dram_tensor("bqkT", [128, 16], dt.float32, kind="ExternalInput")
    bv = nc.dram_tensor("bv", [1, 1024], dt.float32, kind="ExternalInput")
    wp = nc.dram_tensor("wp", [128, 2 * 1024], BF, kind="ExternalInput")
    out_d = nc.dram_tensor("out", [T, C], BF, kind="ExternalOutput")

    with tile.TileContext(nc) as tc:
        with tc.tile_pool(name="persist", bufs=1) as pp, \
             tc.tile_pool(name="drampool", bufs=1, space="DRAM") as dp:
            vscr = [dp.tile([128, 1024], BF, tag=f"vscr{h}",
                            name=f"vscr{h}") for h in range(HPC)]

            xt = pp.tile([128, 8, RPC], BF, tag="xt")
            bqkT_sb = pp.tile([128, 16], dt.float32, tag="bqkT")
            wp_sb = pp.tile([128, 2, 1024], BF, tag="wp")
            mask4 = pp.tile([128, 4, 512], BF, tag="mask4")
            dum = pp.tile([128, 512], BF, tag="dum")
            bv_sb = pp.tile([1, 1024], dt.float32, tag="bv")
            bvb = pp.tile([128, 1024], dt.float32, tag="bvb")

            # f32r, not bf16: matmul cost is identical (1 cyc/row at >=256
            # free) and the device's fp32->bf16 conversion truncates, so
            # SBUF-resident operands stay high-precision for free
            qt_all = pp.tile([64, HPC * T], dt.float32r, tag="qt_all")
            kt_all = pp.tile([64, HPC * T], dt.float32r, tag="kt_all")
            vn = [pp.tile([128, 16 * 65], BF, tag=f"vn{h}", name=f"vn{h}")
                  for h in range(HPC)]            # per-head [V | 1] s-tiles
            yt = [pp.tile([128, T], BF, tag=f"yt{p}", name=f"yt{p}")
                  for p in range(2)]

            # dummy operand first: PE warm-up matmuls depend on it
            nc.vector.memset(dum[:], 0.0)
            # causal mask tiles: mask4[p, d, f] = 1 if f >= 128*d + p else 0
            nc.vector.memset(mask4[:], 1.0)
            for d in range(4):
                nc.gpsimd.affine_select(
                    out=mask4[:, d, :], in_=mask4[:, d, :],
                    compare_op=OP.is_ge, fill=0.0,
                    base=-128 * d, channel_multiplier=-1,
                    pattern=[[1, 512]])
            # ones columns of the [V | 1] tiles (never overwritten by the
            # V gather, which only fills the 64-wide value blocks)
            for h in range(HPC):
                nc.vector.memset(vn[h][:, 64:16 * 65:65], 1.0)

            # attention pools opened early: first S/exp groups are hoisted
            # into phase 1 so ACT warms up while PE finishes the V part
            with tc.tile_pool(name="ptpool", bufs=30) as ptp, \
                 tc.tile_pool(name="ps2", bufs=2, space="PSUM") as ps2:

                def emit_sexp(h, j, sp):
                    """S^T matmuls for an s-pair + exp + causal mask."""
                    ssp = ps2.tile([128, 1024], dt.float32, tag="spsum",
                                   name=f"ssp{h}{j}{sp}")
                    for half in range(2):
                        i = 2 * sp + half
                        nc.tensor.matmul(
                            ssp[:, 512 * half:512 * (half + 1)],
                            kt_all[:, T * h + 128 * i:T * h + 128 * (i + 1)],
                            qt_all[:, T * h + 512 * j:T * h + 512 * (j + 1)],
                            start=True, stop=True)
                    pt = ptp.tile([128, 1024], BF, tag="pt",
                                  name=f"pt{h}{j}{sp}")
                    nc.scalar.activation(pt[:], ssp[:], AF.Exp, scale=SCALE)
                    for half in range(2):
                        i = 2 * sp + half
                        if i >= 4 * j:  # diagonal band: causal mask on DVE
                            nc.vector.tensor_tensor(
                                pt[:, 512 * half:512 * (half + 1)],
                                pt[:, 512 * half:512 * (half + 1)],
                                mask4[:, i - 4 * j, :], op=OP.mult)
                    return pt

                # ================= Phase 1: QKV projection =================
                with tc.tile_pool(name="wstream", bufs=2) as ws, \
                     tc.tile_pool(name="ps1", bufs=2, space="PSUM") as ps1:
                    # --- startup: fine-grained first loads so PE starts
                    # early, plus dummy matmuls to burn through the p-state
                    # ramp while the real inputs stream in ---
                    nc.sync.dma_start(xt[:, 0:1, :], xT[:, 0:1, :])
                    wq0 = ws.tile([128, 8, 128], BF, tag="wq0", bufs=1)
                    nc.sync.dma_start(wq0[:], wq[0].rearrange(
                        "p (k j) -> p k j", k=8))
                    nc.sync.dma_start(xt[:, 1:3, :], xT[:, 1:3, :])
                    nc.sync.dma_start(xt[:, 3:5, :], xT[:, 3:5, :])
                    nc.sync.dma_start(xt[:, 5:8, :], xT[:, 5:8, :])
                    nc.sync.dma_start(bqkT_sb[:], bqkT[:])
                    nc.sync.dma_start(bv_sb[:], bv[:])
                    # v-bias varies with t%16 under the headless reshape, so
                    # it cannot be folded into bproj; add it to the V columns
                    nc.gpsimd.partition_broadcast(bvb[:], bv_sb[:])
                    for w in range(6):
                        psd = ps1.tile([128, RPC], dt.float32, tag="psqk",
                                       name=f"dummy{w}")
                        nc.tensor.matmul(psd[:], dum[:, 0:128], dum[:],
                                         start=True, stop=True)

                    # --- Q,K in transposed orientation: x_proj^T j-tiles ---
                    def qk_evict(m, ps):
                        # evict with bias + stride-16 shuffle into Q^T/K^T
                        dest = qt_all if m < 8 else kt_all
                        gp = 2 * (m % 8)
                        for par in range(2):
                            nc.scalar.activation(
                                dest[:, gp + par:HPC * T:16],
                                ps[64 * par:64 * par + 64, :],
                                AF.Identity, scale=1.0,
                                bias=bqkT_sb[64 * par:64 * par + 64, m:m + 1])

                    ps = ps1.tile([128, RPC], dt.float32, tag="psqk")
                    for k in range(8):
                        nc.tensor.matmul(ps[:], wq0[:, k, :], xt[:, k, :],
                                         start=(k == 0), stop=(k == 7))
                    qk_evict(0, ps)
                    for lo, nm in ((1, 2), (3, 3), (6, 3), (9, 3), (12, 3),
                                   (15, 1)):       # m-tiles 1..15
                        wqt = ws.tile([128, 3, 8, 128], BF, tag="wqt")
                        nc.sync.dma_start(
                            wqt[:, 0:nm], wq[lo:lo + nm].rearrange(
                                "m p (k j) -> p m k j", k=8))
                        for mh in range(nm):
                            m = lo + mh
                            ps = ps1.tile([128, RPC], dt.float32, tag="psqk")
                            for k in range(8):
                                nc.tensor.matmul(ps[:], wqt[:, mh, k, :],
                                                 xt[:, k, :],
                                                 start=(k == 0), stop=(k == 7))
                            qk_evict(m, ps)

                    # hoisted S/exp for (j=3, h=0,1): keeps ACT busy during V
                    hoisted = {(0, sp): emit_sexp(0, 3, sp) for sp in range(8)}
                    hoisted.update({(1, sp): emit_sexp(1, 3, sp) for sp in range(8)})
                    hoisted.update({(2, sp): emit_sexp(2, 3, sp) for sp in range(8)})
                    hoisted.update({(3, sp): emit_sexp(3, 3, sp) for sp in range(4)})

                    # --- V in natural orientation -> DRAM scratch -> per-head
                    # gather back as [s, hd] tiles (re-partition) ---
                    wvt = [None, None]
                    for jv in range(2):
                        wvt[jv] = ws.tile([128, 8, 512], BF, tag="wvt",
                                          bufs=2, name=f"wvt{jv}")
                        nc.sync.dma_start(
                            wvt[jv][:],
                            wv[jv].rearrange("p (k j) -> p k j", k=8))
                    for h in range(HPC):
                        for jv in range(2):
                            ps = ps1.tile([128, 512], dt.float32, tag="psv",
                                          bufs=2)
                            for k in range(8):
                                nc.tensor.matmul(
                                    ps[:], xt[:, k, 128 * h:128 * (h + 1)],
                                    wvt[jv][:, k, :],
                                    start=(k == 0), stop=(k == 7))
                            vsb = ws.tile([128, 512], BF, tag="vsb", bufs=2)
                            nc.vector.tensor_tensor(
                                vsb[:], ps[:],
                                bvb[:, 512 * jv:512 * (jv + 1)], op=OP.add)
                            nc.sync.dma_start(
                                vscr[h][:, 512 * jv:512 * (jv + 1)], vsb[:])
                        # gather this head's V as [s, hd] tiles right away
                        src_ap = vscr[h][:].rearrange(
                            "(i r) (g d) -> (r g) i d", r=8, d=64)
                        dst_ap = vn[h][:].rearrange(
                            "p (i e) -> p i e", e=65)[:, :, 0:64]
                        nc.sync.dma_start(dst_ap, src_ap)

                nc.sync.dma_start(wp_sb[:], wp.rearrange("p (t c) -> p t c", t=2))

                # ===== Phase 2+3: attention (j desc) + fused projection =====
                with tc.tile_pool(name="misc", bufs=2) as mp, \
                     tc.tile_pool(name="osb", bufs=3) as osbp, \
                     tc.tile_pool(name="psy", bufs=2, space="PSUM") as psy, \
                     tc.tile_pool(name="ps3", bufs=2, space="PSUM") as ps3:

                    def emit_pv(h, sp, pt, yps, n_st):
                        for half in range(2):
                            i = 2 * sp + half
                            nc.tensor.matmul(
                                yps[:], vn[h][:, 65 * i:65 * i + 65],
                                pt[:, 512 * half:512 * (half + 1)],
                                start=(i == 0), stop=(i == n_st - 1))

                    def make_norm(h, j, yps):
                        def norm():
                            rec = mp.tile([1, 512], dt.float32, tag="rec",
                                          name=f"rec{h}{j}")
                            with nc.allow_low_precision(reason="softmax recip"):
                                nc.vector.reciprocal(rec[:], yps[64:65, :])
                            bcs = mp.tile([64, 512], dt.float32, tag="bcs",
                                          name=f"bcs{h}{j}")
                            nc.gpsimd.partition_broadcast(bcs[:], rec[:])
                            nc.vector.tensor_tensor(
                                yt[h // 2][64 * (h % 2):64 * (h % 2) + 64,
                                           512 * j:512 * (j + 1)],
                                yps[0:64, :], bcs[:], op=OP.mult)
                        return norm

                    # bproj is added on the host (this kernel emits partial
                    # sums), so the proj eviction is a plain copy — on ACT
                    # for the final t-blocks (ACT is idle in the tail), on
                    # DVE otherwise.  Both column halves of a t-block share
                    # one [128, 1024] ot tile and a single out DMA.
                    ot_tiles = {}

                    def make_proj_one(j, tt, cc, last=False,
                                      evict_act=False):
                        def proj():
                            if last and (tt + cc) % 2 == 0:
                                pw = ps2.tile([128, 1024], dt.float32,
                                              tag="spsum", name=f"pow{tt}{cc}")
                                po = pw[:, 0:512]
                            else:
                                po = ps3.tile([128, 512], dt.float32,
                                              tag="px", name=f"po{tt}{cc}")
                            nc.tensor.matmul(
                                po[:], yt[0][:, 128 * tt:128 * (tt + 1)],
                                wp_sb[:, 0, 512 * cc:512 * (cc + 1)],
                                start=True, stop=False)
                            nc.tensor.matmul(
                                po[:], yt[1][:, 128 * tt:128 * (tt + 1)],
                                wp_sb[:, 1, 512 * cc:512 * (cc + 1)],
                                start=False, stop=True)
                            if tt not in ot_tiles:
                                ot_tiles[tt] = osbp.tile(
                                    [128, 1024], BF, tag="ot", name=f"ot{tt}")
                            dst = ot_tiles[tt][:, 512 * cc:512 * (cc + 1)]
                            if evict_act:
                                nc.scalar.activation(dst, po[:], AF.Copy,
                                                     scale=1.0)
                            else:
                                nc.vector.tensor_copy(dst, po[:])
                            if cc == 1:
                                nc.sync.dma_start(
                                    out_d[128 * tt:128 * (tt + 1), :],
                                    ot_tiles[tt][:])
                        return proj

                    # j order: largest block (j=1) last — its long S/PV
                    # stream hides the drained backlog of j=0, and its own
                    # norms+projs are the only tail
                    pending = []   # small deferred closures, drip-fed
                    reserve = []   # held back to fill the final norm window
                    for jx, j in enumerate([3, 2, 0]):
                        for h in range(HPC):
                            n_st = 4 * j + 4        # s-tiles needed (causal)
                            yps = psy.tile([65, 512], dt.float32, tag="ypsum",
                                           name=f"yps{h}{j}")
                            prev = None
                            for sp in range(n_st // 2):
                                if jx == 0 and (h, sp) in hoisted:
                                    pt = hoisted[(h, sp)]
                                else:
                                    pt = emit_sexp(h, j, sp)
                                if prev is not None:
                                    psp, pt_prev = prev
                                    emit_pv(h, psp, pt_prev, yps, n_st)
                                if sp >= min(2, n_st // 2 - 1):
                                    for _ in range(2 if jx >= 1 else 1):
                                        if pending:
                                            pending.pop(0)()
                                prev = (sp, pt)
                            psp, pt_prev = prev
                            emit_pv(h, psp, pt_prev, yps, n_st)
                            pending.append(make_norm(h, j, yps))
                        for tt in range(4 * j, 4 * j + 4):
                            for cc in range(2):
                                if jx == 2 and tt >= 4 * j + 2:
                                    reserve.append(make_proj_one(
                                        j, tt, cc, last=False,
                                        evict_act=True))
                                else:
                                    pending.append(
                                        make_proj_one(j, tt, cc, last=False))

                    # ---- final block j=1, head pairs interleaved so the two
                    # trailing norm chains overlap ----
                    j, n_st = 1, 8
                    for hp in range(2):
                        ha, hb = 2 * hp, 2 * hp + 1
                        ypsa = psy.tile([65, 512], dt.float32, tag="ypsum",
                                        name=f"ypsA{hp}")
                        ypsb = psy.tile([65, 512], dt.float32, tag="ypsum",
                                        name=f"ypsB{hp}")
                        prev = None
                        for sp in range(n_st // 2):
                            pta = emit_sexp(ha, j, sp)
                            ptb = emit_sexp(hb, j, sp)
                            if prev is not None:
                                psp, pa, pb = prev
                                emit_pv(ha, psp, pa, ypsa, n_st)
                                emit_pv(hb, psp, pb, ypsb, n_st)
                            for _ in range(2):
                                if pending:
                                    pending.pop(0)()
                            prev = (sp, pta, ptb)
                        psp, pa, pb = prev
                        emit_pv(ha, psp, pa, ypsa, n_st)
                        emit_pv(hb, psp, pb, ypsb, n_st)
                        make_norm(ha, j, ypsa)()
                        make_norm(hb, j, ypsb)()
                        if hp == 0:
                            while pending:
                                pending.pop(0)()
                        else:
                            # backfill the norm(2,3) latency window with the
                            # reserved j=0 projections (ACT-evicting, so DVE
                            # stays clear for the final mults)
                            for fn in reserve:
                                fn()

                    # final 8 projections, split: the yt[0] halves run inside
                    # the norm(2,3) latency window (ps2 is free after the
                    # last exp), the yt[1] halves + evictions trail the last
                    # norm.  Evictions ping-pong ACT/DVE; one DMA per t-block.
                    pw = [ps2.tile([128, 1024], dt.float32, tag="spsum",
                                   name=f"pow{i}") for i in range(2)]
                    po_slots = []
                    for idx in range(6):
                        tt, cc = 4 + idx // 2, idx % 2
                        if idx < 4:
                            po = pw[idx // 2][:, 512 * cc:512 * (cc + 1)]
                        else:
                            po = ps3.tile([128, 512], dt.float32, tag="px",
                                          name=f"pol{idx}")
                        nc.tensor.matmul(
                            po[:], yt[0][:, 128 * tt:128 * (tt + 1)],
                            wp_sb[:, 0, 512 * cc:512 * (cc + 1)],
                            start=True, stop=False)
                        po_slots.append((tt, cc, po, False))
                    po_slots.append((7, 0, None, True))
                    po_slots.append((7, 1, None, True))
                    otl = {tt: osbp.tile([128, 1024], BF, tag="otl",
                                         bufs=4, name=f"otl{tt}")
                           for tt in range(4, 8)}
                    for idx, (tt, cc, po, whole) in enumerate(po_slots):
                        if whole:
                            po = ps3.tile([128, 512], dt.float32, tag="px",
                                          name=f"pol{idx}")
                            nc.tensor.matmul(
                                po[:], yt[0][:, 128 * tt:128 * (tt + 1)],
                                wp_sb[:, 0, 512 * cc:512 * (cc + 1)],
                                start=True, stop=False)
                        nc.tensor.matmul(
                            po[:], yt[1][:, 128 * tt:128 * (tt + 1)],
                            wp_sb[:, 1, 512 * cc:512 * (cc + 1)],
                            start=False, stop=True)
                        dst = otl[tt][:, 512 * cc:512 * (cc + 1)]
                        if idx % 2 == 0:
                            nc.scalar.activation(dst, po[:], AF.Copy,
                                                 scale=1.0)
                        else:
                            nc.vector.tensor_copy(dst, po[:])
                        if tt >= 6:
                            nc.sync.dma_start(
                                out_d[128 * tt:128 * (tt + 1),
                                      512 * cc:512 * (cc + 1)], dst)
                        elif cc == 1:
                            nc.sync.dma_start(
                                out_d[128 * tt:128 * (tt + 1), :],
                                otl[tt][:])
    nc.compile()
    return nc


_NC_CACHE = None


def _get_program():
    global _NC_CACHE
    if _NC_CACHE is None:
        _NC_CACHE = build_program()
    return _NC_CACHE


def _bf16(a):
    return np.asarray(a, dtype=np.float32).astype(ml_dtypes.bfloat16)


def _prep_core_inputs(x, Wqkv, bqkv, Wproj, bproj):
    """Build the 8 per-core input dicts (host-side shard + layout prep)."""
    x = np.asarray(x, dtype=np.float32)
    Wqkv = np.ascontiguousarray(np.asarray(Wqkv, dtype=np.float32))
    bqkv = np.asarray(bqkv, dtype=np.float32)
    Wproj = np.asarray(Wproj, dtype=np.float32)
    bproj = np.asarray(bproj, dtype=np.float32)

    wq_np = _bf16(
        Wqkv[:, :2048].reshape(8, 128, 16, 128).transpose(2, 1, 0, 3)
        .reshape(16, 128, 8 * 128))
    wv_np = _bf16(
        Wqkv[:, 2048:].reshape(8, 128, 2, 512).transpose(2, 1, 0, 3)
        .reshape(2, 128, 8 * 512))
    bqkT_np = np.ascontiguousarray(
        bqkv[:2048].reshape(16, 128).T)            # [128, 16] fp32
    bv_np = np.ascontiguousarray(bqkv[2048:].reshape(1, 1024))

    in_maps = []
    for c in range(N_CORES):
        b, q = divmod(c, 4)
        xT_np = _bf16(
            x[b, RPC * q:RPC * (q + 1), :].reshape(RPC, 8, 128)
            .transpose(2, 1, 0))
        wp_np = _bf16(
            Wproj[256 * q:256 * (q + 1), :].reshape(2, 128, 1024)
            .transpose(1, 0, 2).reshape(128, 2048))
        in_maps.append({
            "xT": xT_np, "wq": wq_np, "wv": wv_np, "bqkT": bqkT_np,
            "bv": bv_np, "wp": wp_np,
        })
    return in_maps


def kernel(x, Wqkv, bqkv, Wproj, bproj):
    nc = _get_program()
    in_maps = _prep_core_inputs(x, Wqkv, bqkv, Wproj, bproj)
    res = run_bass_kernel_spmd(nc, in_maps, list(range(N_CORES)))
    # device emits partial sums; the projection bias is added here
    bias = np.asarray(bproj, np.float32)
    out = np.zeros((B, T, C), dtype=np.float32)
    for c in range(N_CORES):
        out[c // 4] += np.asarray(res.results[c]["out"], dtype=np.float32)
    out += bias
    return out



# revision 33
# speedup vs baseline: 1.0767x; 1.0767x over previous
"""Causal self-attention (dense transformer) on 8 trn2 NeuronCores — v3.

Same core sharding as v1 (batch x row-quarter/head-group per core, host-side
row-parallel reduction of the output projection), with:
- bf16 matmul operands on every DMA-touched tensor (x, weights, V bounce,
  y, output partials); SBUF-resident Q^T/K^T stay f32r, which costs the
  same on the PE (1 cyc/row at >=256 free) but avoids the device's
  truncating fp32->bf16 conversions.  End-to-end error ~3.4e-3.
- QK bias folded into the PSUM eviction on ACT (Identity+bias, per-partition
  bias AP) instead of PE bias matmuls
- V bias added during the V eviction (tensor_tensor add against a
  partition-broadcast bias tile).  NOTE: it cannot be folded into bproj --
  under the headless reshape the v-bias seen by (t, d) is
  bqkv[2048 + 64*(t%16) + d], which varies across the keys averaged by the
  softmax.
- projection bias added on the host (the device emits partial sums)
- softmax denominator reciprocal read straight out of PSUM on DVE; the
  [1,512] -> [64,512] broadcast on Pool (partition_broadcast) instead of a
  PE outer product
- causal masks as 4 precomputed bf16 0/1 tiles (one per diagonal offset),
  applied on DVE (tensor_tensor mult, all-SBUF 2x mode) instead of
  gpsimd affine_select per tile
- V re-partition bounce through DRAM in bf16 with per-head gathers issued
  as soon as each head's V columns are evicted; ones column via memset
- phase 2 runs j blocks in order [3, 2, 0, 1]: the largest block last gives
  the longest S/PV stream to hide the drained closure backlog, its head
  pairs are interleaved so the two trailing norm chains overlap, and the
  final projections split their yt[0] halves into the norm-latency window
- PE warm-up dummy matmuls burn the p-state ramp during the first loads

Shapes (hardcoded): B=2, T=2048, C=1024, n_head=16, hd=64, 8 cores.
Core c: batch b=c//4, quarter q=c%4 -> x rows [512q, 512q+512), heads 4q..4q+3.
"""

import os

import numpy as np
import ml_dtypes

os.environ.setdefault("NEURON_RT_RESET_CORES", "1")

import concourse.bacc as bacc
import concourse.mybir as mybir
import concourse.tile as tile
from concourse.bass_utils import run_bass_kernel_spmd

dt = mybir.dt
AF = mybir.ActivationFunctionType
OP = mybir.AluOpType

B, T, C = 2, 2048, 1024
NH, HD = 16, 64
N_CORES = 8
HPC = 4          # heads per core
RPC = 512        # x rows per core
SCALE = 1.0 / 8.0   # 1/sqrt(hd), folded into the exp activation
BF = dt.bfloat16


def build_program():
    nc = bacc.Bacc("TRN2", target_bir_lowering=False, debug=False,
                   num_devices=N_CORES)

    # ---- DRAM I/O (per core) ----
    xT = nc.dram_tensor("xT", [128, 8, RPC], BF, kind="ExternalInput")
    # wq partition-major: any m-tile slice [:, lo:lo+nm] is contiguous
    # (nm*2KB) per partition -- big DMA elements, half the descriptors
    wq = nc.dram_tensor("wq", [128, 16, 8 * 128], BF, kind="ExternalInput")
    wv = nc.dram_tensor("wv", [2, 128, 8 * 512], BF, kind="ExternalInput")
    bqkT = nc.dram_tensor("bqkT", [128, 16], dt.float32, kind="ExternalInput")
    bv = nc.dram_tensor("bv", [1, 1024], dt.float32, kind="ExternalInput")
    wp = nc.dram_tensor("wp", [128, 2 * 1024], BF, kind="ExternalInput")
    out_d = nc.dram_tensor("out", [T, C], BF, kind="ExternalOutput")

    with tile.TileContext(nc) as tc:
        with tc.tile_pool(name="persist", bufs=1) as pp, \
             tc.tile_pool(name="drampool", bufs=1, space="DRAM") as dp:
            vscr = [dp.tile([128, 1024], BF, tag=f"vscr{h}",
                            name=f"vscr{h}") for h in range(HPC)]

            xt = pp.tile([128, 8, RPC], BF, tag="xt")
            bqkT_sb = pp.tile([128, 16], dt.float32, tag="bqkT")
            wp_sb = pp.tile([128, 2, 1024], BF, tag="wp")
            tri = pp.tile([128, 128], BF, tag="tri")
            dum = pp.tile([128, 512], BF, tag="dum")
            bv_sb = pp.tile([1, 1024], dt.float32, tag="bv")
            bvb = pp.tile([128, 1024], dt.float32, tag="bvb")

            # f32r, not bf16: matmul cost is identical (1 cyc/row at >=256
            # free) and the device's fp32->bf16 conversion truncates, so
            # SBUF-resident operands stay high-precision for free
            qt_all = pp.tile([64, HPC * T], dt.float32r, tag="qt_all")
            kt_all = pp.tile([64, HPC * T], dt.float32r, tag="kt_all")
            vn = [pp.tile([128, 16 * 65], BF, tag=f"vn{h}", name=f"vn{h}")
                  for h in range(HPC)]            # per-head [V | 1] s-tiles
            yt = [pp.tile([128, T], BF, tag=f"yt{p}", name=f"yt{p}")
                  for p in range(2)]

            # dummy operand first (on Pool: its preamble clears ~700ns before
            # DVE's, so PE warm-up starts earlier)
            nc.gpsimd.memset(dum[:], 0.0)
            # shared causal triangle: tri[p, g] = 1 if g >= p else 0; every
            # diagonal s-tile uses the same [128,128] block mask
            nc.vector.memset(tri[:], 1.0)
            nc.gpsimd.affine_select(
                out=tri[:], in_=tri[:], compare_op=OP.is_ge, fill=0.0,
                base=0, channel_multiplier=-1, pattern=[[1, 128]])
            # ones columns of the [V | 1] tiles (never overwritten by the
            # V gather, which only fills the 64-wide value blocks)
            for h in range(HPC):
                nc.vector.memset(vn[h][:, 64:16 * 65:65], 1.0)

            # attention pools opened early: first S/exp groups are hoisted
            # into phase 1 so ACT warms up while PE finishes the V part
            with tc.tile_pool(name="ptpool", bufs=36) as ptp, \
                 tc.tile_pool(name="ps2", bufs=2, space="PSUM") as ps2:

                def emit_sexp(h, j, sp):
                    """S^T matmuls for an s-pair + exp + causal mask.

                    Diagonal s-tiles (d = i-4j >= 0) only need queries
                    w >= 128d: the S matmul is trimmed to that range (floored
                    at 256 wide -- f32r matmuls under 256 free run at 1/4
                    rate), the exp starts at the first half's trim, and the
                    mask shrinks to the [128d, 128d+128) triangular block
                    (columns below 128d are simply never read by the trimmed
                    PV matmuls)."""
                    ssp = ps2.tile([128, 1024], dt.float32, tag="spsum",
                                   name=f"ssp{h}{j}{sp}")
                    explo = 0
                    for half in range(2):
                        i = 2 * sp + half
                        d = i - 4 * j
                        qlo = 0 if d <= 0 else min(128 * d, 256)
                        if half == 0:
                            explo = qlo
                        nc.tensor.matmul(
                            ssp[:, 512 * half + qlo:512 * (half + 1)],
                            kt_all[:, T * h + 128 * i:T * h + 128 * (i + 1)],
                            qt_all[:, T * h + 512 * j + qlo:
                                   T * h + 512 * (j + 1)],
                            start=True, stop=True)
                    pt = ptp.tile([128, 1024], BF, tag="pt",
                                  name=f"pt{h}{j}{sp}")
                    if explo == 256:
                        # second diagonal pair (d=2,3): the two needed spans
                        # [256:512] and [896:1024] are narrow enough that two
                        # exps (extra init overhead) beat one wide exp
                        nc.scalar.activation(pt[:, 256:512], ssp[:, 256:512],
                                             AF.Exp, scale=SCALE)
                        nc.scalar.activation(pt[:, 896:1024], ssp[:, 896:1024],
                                             AF.Exp, scale=SCALE)
                    else:
                        nc.scalar.activation(pt[:, explo:1024],
                                             ssp[:, explo:1024],
                                             AF.Exp, scale=SCALE)
                    for half in range(2):
                        i = 2 * sp + half
                        d = i - 4 * j
                        if d >= 0:  # diagonal band: triangle mask on DVE
                            lo = 512 * half + 128 * d
                            nc.vector.tensor_tensor(
                                pt[:, lo:lo + 128], pt[:, lo:lo + 128],
                                tri[:], op=OP.mult)
                    return pt

                # ================= Phase 1: QKV projection =================
                with tc.tile_pool(name="wstream", bufs=2) as ws, \
                     tc.tile_pool(name="ps1", bufs=2, space="PSUM") as ps1:
                    # --- startup: fine-grained first loads so PE starts
                    # early, plus dummy matmuls to keep the PE ramp warm
                    # while the real inputs stream in (idle gaps >~1us can
                    # reset the p-state ramp) ---
                    wq0 = ws.tile([128, 8, 128], BF, tag="wq0", bufs=1)
                    nc.sync.dma_start(wq0[:], wq[:, 0:1].rearrange(
                        "p m (k j) -> p (m k) j", k=8))
                    nc.sync.dma_start(xt[:, 0:2, :], xT[:, 0:2, :])
                    nc.sync.dma_start(xt[:, 2:5, :], xT[:, 2:5, :])
                    nc.sync.dma_start(xt[:, 5:8, :], xT[:, 5:8, :])
                    nc.sync.dma_start(bqkT_sb[:], bqkT[:])
                    nc.sync.dma_start(bv_sb[:], bv[:])
                    # v-bias varies with t%16 under the headless reshape, so
                    # it cannot be folded into bproj; add it to the V columns
                    nc.gpsimd.partition_broadcast(bvb[:], bv_sb[:])
                    for w in range(6):
                        psd = ps1.tile([128, RPC], dt.float32, tag="psqk",
                                       name=f"dummy{w}")
                        nc.tensor.matmul(psd[:], dum[:, 0:128], dum[:],
                                         start=True, stop=True)

                    # --- Q,K in transposed orientation: x_proj^T j-tiles ---
                    def qk_evict(m, ps):
                        # evict with bias + stride-16 shuffle into Q^T/K^T.
                        # On DVE (tensor_scalar, per-partition bias AP): ACT
                        # is the attention-phase bottleneck (softmax exps),
                        # so Q/K evictions must stay off it.
                        dest = qt_all if m < 8 else kt_all
                        gp = 2 * (m % 8)
                        for par in range(2):
                            nc.vector.tensor_scalar(
                                dest[:, gp + par:HPC * T:16],
                                ps[64 * par:64 * par + 64, :],
                                bqkT_sb[64 * par:64 * par + 64, m:m + 1],
                                None, OP.add)

                    ps = ps1.tile([128, RPC], dt.float32, tag="psqk")
                    for k in range(8):
                        nc.tensor.matmul(ps[:], wq0[:, k, :], xt[:, k, :],
                                         start=(k == 0), stop=(k == 7))
                    qk_evict(0, ps)
                    for lo, nm in ((1, 1), (2, 2), (4, 3), (7, 3), (10, 3),
                                   (13, 3)):       # m-tiles 1..15
                        wqt = ws.tile([128, 3, 8, 128], BF, tag="wqt")
                        nc.sync.dma_start(
                            wqt[:, 0:nm], wq[:, lo:lo + nm].rearrange(
                                "p m (k j) -> p m k j", k=8))
                        for mh in range(nm):
                            m = lo + mh
                            ps = ps1.tile([128, RPC], dt.float32, tag="psqk")
                            for k in range(8):
                                nc.tensor.matmul(ps[:], wqt[:, mh, k, :],
                                                 xt[:, k, :],
                                                 start=(k == 0), stop=(k == 7))
                            qk_evict(m, ps)

                    # hoisted S/exp groups: phase-1 ACT is otherwise idle
                    # (evictions live on DVE), so pre-compute as many exps
                    # as the pt pool can hold -- phase 2's per-group cadence
                    # is ACT-bound, and every hoisted group removes one exp
                    # from that critical chain.  Emission order == phase-2
                    # consumption order (FIFO pool).
                    hoisted = {}
                    for h in range(HPC):
                        for sp in range(8):
                            hoisted[(h, 3, sp)] = emit_sexp(h, 3, sp)
                    for sp in range(4):
                        hoisted[(0, 2, sp)] = emit_sexp(0, 2, sp)

                    # --- V in natural orientation -> DRAM scratch -> per-head
                    # gather back as [s, hd] tiles (re-partition) ---
                    wvt = [None, None]
                    for jv in range(2):
                        wvt[jv] = ws.tile([128, 8, 512], BF, tag="wvt",
                                          bufs=2, name=f"wvt{jv}")
                        nc.sync.dma_start(
                            wvt[jv][:],
                            wv[jv].rearrange("p (k j) -> p k j", k=8))
                    for h in range(HPC):
                        for jv in range(2):
                            ps = ps1.tile([128, 512], dt.float32, tag="psv",
                                          bufs=2)
                            for k in range(8):
                                nc.tensor.matmul(
                                    ps[:], xt[:, k, 128 * h:128 * (h + 1)],
                                    wvt[jv][:, k, :],
                                    start=(k == 0), stop=(k == 7))
                            vsb = ws.tile([128, 512], BF, tag="vsb", bufs=2)
                            nc.vector.tensor_tensor(
                                vsb[:], ps[:],
                                bvb[:, 512 * jv:512 * (jv + 1)], op=OP.add)
                            nc.sync.dma_start(
                                vscr[h][:, 512 * jv:512 * (jv + 1)], vsb[:])
                        # gather this head's V as [s, hd] tiles right away
                        src_ap = vscr[h][:].rearrange(
                            "(i r) (g d) -> (r g) i d", r=8, d=64)
                        dst_ap = vn[h][:].rearrange(
                            "p (i e) -> p i e", e=65)[:, :, 0:64]
                        nc.sync.dma_start(dst_ap, src_ap)

                nc.sync.dma_start(wp_sb[:], wp.rearrange("p (t c) -> p t c", t=2))

                # ===== Phase 2+3: attention (j desc) + fused projection =====
                with tc.tile_pool(name="misc", bufs=2) as mp, \
                     tc.tile_pool(name="osb", bufs=3) as osbp, \
                     tc.tile_pool(name="psy", bufs=2, space="PSUM") as psy, \
                     tc.tile_pool(name="ps3", bufs=2, space="PSUM") as ps3:

                    def emit_pv(h, j, sp, pt, yps, n_st):
                        # diagonal s-tiles contribute nothing to queries
                        # w < 128d -- trim the moving dim to [128d, 512)
                        for half in range(2):
                            i = 2 * sp + half
                            d = i - 4 * j
                            plo = 0 if d <= 0 else 128 * d
                            nc.tensor.matmul(
                                yps[:, plo:512], vn[h][:, 65 * i:65 * i + 65],
                                pt[:, 512 * half + plo:512 * (half + 1)],
                                start=(i == 0), stop=(i == n_st - 1))

                    def make_norm(h, j, yps):
                        def norm():
                            rec = mp.tile([1, 512], dt.float32, tag="rec",
                                          name=f"rec{h}{j}")
                            with nc.allow_low_precision(reason="softmax recip"):
                                nc.vector.reciprocal(rec[:], yps[64:65, :])
                            bcs = mp.tile([64, 512], dt.float32, tag="bcs",
                                          name=f"bcs{h}{j}")
                            nc.gpsimd.partition_broadcast(bcs[:], rec[:])
                            nc.vector.tensor_tensor(
                                yt[h // 2][64 * (h % 2):64 * (h % 2) + 64,
                                           512 * j:512 * (j + 1)],
                                yps[0:64, :], bcs[:], op=OP.mult)
                        return norm

                    # bproj is added on the host (this kernel emits partial
                    # sums), so the proj eviction is a plain copy — on ACT
                    # for the final t-blocks (ACT is idle in the tail), on
                    # DVE otherwise.  Both column halves of a t-block share
                    # one [128, 1024] ot tile and a single out DMA.
                    ot_tiles = {}

                    def make_proj_one(j, tt, cc, last=False,
                                      evict_act=False):
                        def proj():
                            if last and (tt + cc) % 2 == 0:
                                pw = ps2.tile([128, 1024], dt.float32,
                                              tag="spsum", name=f"pow{tt}{cc}")
                                po = pw[:, 0:512]
                            else:
                                po = ps3.tile([128, 512], dt.float32,
                                              tag="px", name=f"po{tt}{cc}")
                            nc.tensor.matmul(
                                po[:], yt[0][:, 128 * tt:128 * (tt + 1)],
                                wp_sb[:, 0, 512 * cc:512 * (cc + 1)],
                                start=True, stop=False)
                            nc.tensor.matmul(
                                po[:], yt[1][:, 128 * tt:128 * (tt + 1)],
                                wp_sb[:, 1, 512 * cc:512 * (cc + 1)],
                                start=False, stop=True)
                            if tt not in ot_tiles:
                                ot_tiles[tt] = osbp.tile(
                                    [128, 1024], BF, tag="ot", name=f"ot{tt}")
                            dst = ot_tiles[tt][:, 512 * cc:512 * (cc + 1)]
                            if evict_act:
                                nc.scalar.activation(dst, po[:], AF.Copy,
                                                     scale=1.0)
                            else:
                                nc.vector.tensor_copy(dst, po[:])
                            if cc == 1:
                                nc.sync.dma_start(
                                    out_d[128 * tt:128 * (tt + 1), :],
                                    ot_tiles[tt][:])
                        return proj

                    # j order: largest block (j=1) last — its long S/PV
                    # stream hides the drained backlog of j=0, and its own
                    # norms+projs are the only tail
                    pending = []   # small deferred closures, drip-fed
                    pnorms = []    # norm closures: pop first (psy rotation)
                    reserve = []   # held back to fill the final norm window

                    def pop_closures(n):
                        if pnorms:
                            pnorms.pop(0)()
                        for _ in range(n):
                            if pending:
                                pending.pop(0)()
                    for jx, j in enumerate([3, 2]):
                        for h in range(HPC):
                            n_st = 4 * j + 4        # s-tiles needed (causal)
                            yps = psy.tile([65, 512], dt.float32, tag="ypsum",
                                           name=f"yps{h}{j}")
                            prev = None
                            for sp in range(n_st // 2):
                                if (h, j, sp) in hoisted:
                                    pt = hoisted.pop((h, j, sp))
                                else:
                                    pt = emit_sexp(h, j, sp)
                                if prev is not None:
                                    psp, pt_prev = prev
                                    emit_pv(h, j, psp, pt_prev, yps, n_st)
                                if sp >= min(2, n_st // 2 - 1):
                                    pop_closures(2 if jx >= 1 else 1)
                                prev = (sp, pt)
                            psp, pt_prev = prev
                            emit_pv(h, j, psp, pt_prev, yps, n_st)
                            pnorms.append(make_norm(h, j, yps))
                        for tt in range(4 * j, 4 * j + 4):
                            for cc in range(2):
                                pending.append(
                                    make_proj_one(j, tt, cc, last=False))

                    # ---- j=0 block: interleaved head pairs (each head has
                    # only ~1.1us of PE work, so a lone head stalls on exp
                    # latency; its partner's matmuls fill the gap) ----
                    j, n_st = 0, 4
                    for hp in range(2):
                        ha, hb = 2 * hp, 2 * hp + 1
                        ypsa = psy.tile([65, 512], dt.float32, tag="ypsum",
                                        name=f"yps0A{hp}")
                        ypsb = psy.tile([65, 512], dt.float32, tag="ypsum",
                                        name=f"yps0B{hp}")
                        prev = None
                        for sp in range(n_st // 2):
                            pta = emit_sexp(ha, j, sp)
                            ptb = emit_sexp(hb, j, sp)
                            if prev is not None:
                                psp, pa, pb = prev
                                emit_pv(ha, j, psp, pa, ypsa, n_st)
                                emit_pv(hb, j, psp, pb, ypsb, n_st)
                            pop_closures(2)
                            prev = (sp, pta, ptb)
                        psp, pa, pb = prev
                        emit_pv(ha, j, psp, pa, ypsa, n_st)
                        emit_pv(hb, j, psp, pb, ypsb, n_st)
                        pnorms.append(make_norm(ha, j, ypsa))
                        pnorms.append(make_norm(hb, j, ypsb))
                    for tt in range(0, 4):
                        for cc in range(2):
                            if tt >= 2:
                                reserve.append(make_proj_one(
                                    j, tt, cc, last=False, evict_act=True))
                            else:
                                pending.append(
                                    make_proj_one(j, tt, cc, last=False))

                    # ---- final block j=1, head pairs interleaved so the two
                    # trailing norm chains overlap ----
                    j, n_st = 1, 8
                    for hp in range(2):
                        ha, hb = 2 * hp, 2 * hp + 1
                        ypsa = psy.tile([65, 512], dt.float32, tag="ypsum",
                                        name=f"ypsA{hp}")
                        ypsb = psy.tile([65, 512], dt.float32, tag="ypsum",
                                        name=f"ypsB{hp}")
                        prev = None
                        for sp in range(n_st // 2):
                            pta = emit_sexp(ha, j, sp)
                            ptb = emit_sexp(hb, j, sp)
                            if prev is not None:
                                psp, pa, pb = prev
                                emit_pv(ha, j, psp, pa, ypsa, n_st)
                                emit_pv(hb, j, psp, pb, ypsb, n_st)
                            pop_closures(2)
                            prev = (sp, pta, ptb)
                        psp, pa, pb = prev
                        emit_pv(ha, j, psp, pa, ypsa, n_st)
                        emit_pv(hb, j, psp, pb, ypsb, n_st)
                        if hp == 0:
                            while pnorms:
                                pnorms.pop(0)()
                            make_norm(ha, j, ypsa)()
                            make_norm(hb, j, ypsb)()
                            while pending:
                                pending.pop(0)()
                        else:
                            yps_fin = (ypsa, ypsb)

                    # ---- tail: the hp=1 norms split into two 256-query
                    # chunks so t-blocks 4,5 project and DMA out while the
                    # 768..1024 chunk is still normalizing.  The reserved
                    # j=0 projections + the yt[0]-half matmuls fill the PE
                    # through the norm latency. ----
                    for fn in reserve:
                        fn()
                    pw = [ps2.tile([128, 1024], dt.float32, tag="spsum",
                                   name=f"pow{i}") for i in range(2)]
                    po_map = {}
                    for idx in range(6):
                        tt, cc = 4 + idx // 2, idx % 2
                        if idx < 4:
                            po = pw[idx // 2][:, 512 * cc:512 * (cc + 1)]
                        else:
                            po = ps3.tile([128, 512], dt.float32, tag="px",
                                          name=f"pol{idx}")
                        nc.tensor.matmul(
                            po[:], yt[0][:, 128 * tt:128 * (tt + 1)],
                            wp_sb[:, 0, 512 * cc:512 * (cc + 1)],
                            start=True, stop=False)
                        po_map[(tt, cc)] = po
                    otl = {tt: osbp.tile([128, 1024], BF, tag="otl",
                                         bufs=4, name=f"otl{tt}")
                           for tt in range(4, 8)}
                    # all four norm chunks first: DVE runs the recips+mults
                    # back-to-back (Pool pipelines the broadcasts) so every
                    # yt[1] column is ready before the eviction traffic
                    # lands on DVE
                    for ci in range(2):
                        for hx, yp in enumerate(yps_fin):
                            rec = mp.tile([1, 256], dt.float32, tag="rec2",
                                          name=f"recs{hx}{ci}")
                            with nc.allow_low_precision(reason="softmax recip"):
                                nc.vector.reciprocal(
                                    rec[:], yp[64:65, 256 * ci:256 * (ci + 1)])
                            bcs = mp.tile([64, 256], dt.float32, tag="bcs2",
                                          name=f"bcss{hx}{ci}")
                            nc.gpsimd.partition_broadcast(bcs[:], rec[:])
                            nc.vector.tensor_tensor(
                                yt[1][64 * hx:64 * hx + 64,
                                      512 + 256 * ci:512 + 256 * (ci + 1)],
                                yp[0:64, 256 * ci:256 * (ci + 1)], bcs[:],
                                op=OP.mult)
                    # finals: evict cc=0 on DVE / cc=1 on ACT in parallel,
                    # one DMA per t-block; tt=7 as two 512-wide chunks so
                    # the very last DMA is short
                    for tt in range(4, 8):
                        for cc in range(2):
                            po = po_map.get((tt, cc))
                            if po is None:
                                po = ps3.tile([128, 512], dt.float32,
                                              tag="px", name=f"pw{tt}{cc}")
                                nc.tensor.matmul(
                                    po[:],
                                    yt[0][:, 128 * tt:128 * (tt + 1)],
                                    wp_sb[:, 0, 512 * cc:512 * (cc + 1)],
                                    start=True, stop=False)
                            nc.tensor.matmul(
                                po[:], yt[1][:, 128 * tt:128 * (tt + 1)],
                                wp_sb[:, 1, 512 * cc:512 * (cc + 1)],
                                start=False, stop=True)
                            dst = otl[tt][:, 512 * cc:512 * (cc + 1)]
                            if cc == 1:
                                nc.scalar.activation(dst, po[:], AF.Copy,
                                                     scale=1.0)
                            else:
                                nc.vector.tensor_copy(dst, po[:])
                            if tt == 7:
                                nc.sync.dma_start(
                                    out_d[896:1024,
                                          512 * cc:512 * (cc + 1)], dst)
                            elif cc == 1:
                                nc.sync.dma_start(
                                    out_d[128 * tt:128 * (tt + 1), :],
                                    otl[tt][:])
    nc.compile()
    return nc


_NC_CACHE = None


def _get_program():
    global _NC_CACHE
    if _NC_CACHE is None:
        _NC_CACHE = build_program()
    return _NC_CACHE


def _bf16(a):
    return np.asarray(a, dtype=np.float32).astype(ml_dtypes.bfloat16)


def _prep_core_inputs(x, Wqkv, bqkv, Wproj, bproj):
    """Build the 8 per-core input dicts (host-side shard + layout prep)."""
    x = np.asarray(x, dtype=np.float32)
    Wqkv = np.ascontiguousarray(np.asarray(Wqkv, dtype=np.float32))
    bqkv = np.asarray(bqkv, dtype=np.float32)
    Wproj = np.asarray(Wproj, dtype=np.float32)
    bproj = np.asarray(bproj, dtype=np.float32)

    wq_np = _bf16(
        Wqkv[:, :2048].reshape(8, 128, 16, 128).transpose(1, 2, 0, 3)
        .reshape(128, 16, 8 * 128))
    wv_np = _bf16(
        Wqkv[:, 2048:].reshape(8, 128, 2, 512).transpose(2, 1, 0, 3)
        .reshape(2, 128, 8 * 512))
    bqkT_np = np.ascontiguousarray(
        bqkv[:2048].reshape(16, 128).T)            # [128, 16] fp32
    bv_np = np.ascontiguousarray(bqkv[2048:].reshape(1, 1024))

    in_maps = []
    for c in range(N_CORES):
        b, q = divmod(c, 4)
        xT_np = _bf16(
            x[b, RPC * q:RPC * (q + 1), :].reshape(RPC, 8, 128)
            .transpose(2, 1, 0))
        wp_np = _bf16(
            Wproj[256 * q:256 * (q + 1), :].reshape(2, 128, 1024)
            .transpose(1, 0, 2).reshape(128, 2048))
        in_maps.append({
            "xT": xT_np, "wq": wq_np, "wv": wv_np, "bqkT": bqkT_np,
            "bv": bv_np, "wp": wp_np,
        })
    return in_maps


def kernel(x, Wqkv, bqkv, Wproj, bproj):
    nc = _get_program()
    in_maps = _prep_core_inputs(x, Wqkv, bqkv, Wproj, bproj)
    res = run_bass_kernel_spmd(nc, in_maps, list(range(N_CORES)))
    # device emits partial sums; the projection bias is added here
    bias = np.asarray(bproj, np.float32)
    out = np.zeros((B, T, C), dtype=np.float32)
    for c in range(N_CORES):
        out[c // 4] += np.asarray(res.results[c]["out"], dtype=np.float32)
    out += bias
    return out

